# revision 1
# baseline (speedup 1.0000x reference)
"""GraphSAGE 2-layer GNN on TRN2, 8-core SPMD Bass/Tile kernel.

Strategy:
- Nodes sharded across 8 cores (6250 each). Edges partitioned by dst core.
- x replicated in each core's HBM; per-edge neighbor features fetched with
  dma_gather (512B rows, ~HBM line rate).
- Segment-sum via one-hot matmul on PE: edges sorted by dst tile; for each
  128-edge chunk, onehot[e,d] = (dstloc[e]==d) built on DVE, then
  psum[dst,feat] += onehot.T @ msgs.
- int16 gather indices => split edges into low (src<32768) / high streams.
- Layer 2 gathers z = h @ Wl2 (padded to 128 bf16 cols = 256B rows) after an
  8-rank AllGather of per-core z slices.
"""
from dataclasses import dataclass, field
import numpy as np
import ml_dtypes

import concourse.bacc as bacc
import concourse.bass as bass
import concourse.mybir as mybir
import concourse.tile as tile
from concourse import library_config

P = 128


@dataclass
class Plan:
    n_nodes: int
    n_feat: int
    n_hid: int
    n_class: int
    n_cores: int
    npc: int                 # nodes per core
    nt: int                  # dst tiles per core
    win: int                 # gather window (slots per dma_gather call)
    kb: int                  # chunks per one-hot build
    split: int               # low/high src index split (int16 limit)
    budget: np.ndarray       # [nt, 2] chunks per (tile, group)
    nl: int = 0              # total slots, low stream
    nh: int = 0              # total slots, high stream
    # per-core staged arrays
    idx_lo: list = field(default_factory=list)   # [128, nl/16] int16
    idx_hi: list = field(default_factory=list)
    dst_lo: list = field(default_factory=list)   # [128, nl/128] f32
    dst_hi: list = field(default_factory=list)
    invc_tiled: list = field(default_factory=list)  # [128, nt] f32
    xT_own: list = field(default_factory=list)      # [128, nt*128] f32


def _wrap_idx(arr_i16: np.ndarray) -> np.ndarray:
    # position j -> partition j%16, col j//16; replicated 8x down partitions
    w = arr_i16.reshape(-1, 16).T            # [16, n/16]
    return np.ascontiguousarray(np.tile(w, (8, 1)))  # [128, n/16]


def _wrap_slots(arr_f32: np.ndarray) -> np.ndarray:
    # position j -> partition j%128, col j//128 (matches dma_gather output)
    return np.ascontiguousarray(arr_f32.reshape(-1, P).T)  # [128, n/128]


def make_plan(edge_index: np.ndarray, n_nodes: int, n_feat: int, n_hid: int,
              n_class: int, n_cores: int, win: int = 1024, kb: int = 8,
              split: int = 32768) -> Plan:
    src = np.asarray(edge_index[0], dtype=np.int64)
    dst = np.asarray(edge_index[1], dtype=np.int64)
    npc = n_nodes // n_cores
    assert npc * n_cores == n_nodes
    nt = (npc + P - 1) // P

    deg = np.bincount(dst, minlength=n_nodes).astype(np.float64)
    invc = (1.0 / np.maximum(deg, 1.0)).astype(np.float32)

    core_of = dst // npc
    tloc = (dst - core_of * npc) // P          # 0..nt-1
    grp = (src >= split).astype(np.int64)      # 0=low, 1=high

    # counts[c, t, g]
    counts = np.zeros((n_cores, nt, 2), dtype=np.int64)
    np.add.at(counts, (core_of, tloc, grp), 1)
    budget = np.ceil(counts.max(axis=0) / P).astype(np.int64)  # [nt, 2] chunks

    plan = Plan(n_nodes=n_nodes, n_feat=n_feat, n_hid=n_hid, n_class=n_class,
                n_cores=n_cores, npc=npc, nt=nt, win=win, kb=kb, split=split,
                budget=budget)
    nl = int(budget[:, 0].sum()) * P
    nh = int(budget[:, 1].sum()) * P
    plan.nl, plan.nh = nl, nh

    # slot offsets per (t, g) within each stream
    off_l = np.concatenate([[0], np.cumsum(budget[:, 0])])[:-1] * P
    off_h = np.concatenate([[0], np.cumsum(budget[:, 1])])[:-1] * P
    plan.off_l, plan.off_h = off_l, off_h

    order = np.argsort(core_of * (nt * 2) + tloc * 2 + grp, kind="stable")
    for c in range(n_cores):
        idxs = {0: np.zeros(nl, np.int16), 1: np.zeros(nh, np.int16)}
        dsts = {0: np.full(nl, -1.0, np.float32), 1: np.full(nh, -1.0, np.float32)}
        offs = {0: off_l, 1: off_h}
        sel = order[np.searchsorted(core_of[order], c, side="left"):
                    np.searchsorted(core_of[order], c, side="right")]
        # sel is sorted by (t, g); group contiguous runs
        st = src[sel]
        dt_ = dst[sel]
        tl = tloc[sel]
        gl = grp[sel]
        key = tl * 2 + gl
        boundaries = np.concatenate([[0], np.where(np.diff(key) != 0)[0] + 1, [len(sel)]])
        for b0, b1 in zip(boundaries[:-1], boundaries[1:]):
            t = int(tl[b0]); g = int(gl[b0])
            n = b1 - b0
            o = int(offs[g][t])
            s_ids = st[b0:b1]
            idxs[g][o:o + n] = (s_ids - (split if g else 0)).astype(np.int16)
            dsts[g][o:o + n] = (dt_[b0:b1] - c * npc - t * P).astype(np.float32)
        plan.idx_lo.append(_wrap_idx(idxs[0]))
        plan.idx_hi.append(_wrap_idx(idxs[1]))
        plan.dst_lo.append(_wrap_slots(dsts[0]))
        plan.dst_hi.append(_wrap_slots(dsts[1]))
        ic = np.zeros((P, nt), np.float32)
        base = c * npc
        n_own = npc
        icl = invc[base:base + n_own]
        icl = np.concatenate([icl, np.zeros(nt * P - n_own, np.float32)])
        ic[:, :] = icl.reshape(nt, P).T
        plan.invc_tiled.append(np.ascontiguousarray(ic))
    return plan


def stage_inputs(plan: Plan, x, Wl1, Wr1, b1, Wl2, Wr2, b2):
    """Build per-core in_maps (numpy) for the bass program."""
    n, f = x.shape
    hid = plan.n_hid
    ncl = plan.n_class
    zcols = P  # bf16 z row padded to 128 cols = 256B
    x_f32 = np.ascontiguousarray(np.asarray(x, dtype=np.float32))
    wl1 = np.ascontiguousarray(np.asarray(Wl1, np.float32))
    wr1 = np.ascontiguousarray(np.asarray(Wr1, np.float32))
    wl2p = np.zeros((hid, zcols), np.float32)
    wl2p[:, :ncl] = np.asarray(Wl2, np.float32)
    wr2 = np.ascontiguousarray(np.asarray(Wr2, np.float32))
    b1c = np.asarray(b1, np.float32).reshape(hid, 1)
    b2bc = np.broadcast_to(np.asarray(b2, np.float32), (P, ncl)).copy()
    iota = np.broadcast_to(np.arange(P, dtype=np.float32), (P, P)).copy()
    ident = np.eye(P, dtype=np.float32)

    in_maps = []
    for c in range(plan.n_cores):
        base = c * plan.npc
        xt = np.zeros((P, plan.nt * P), np.float32)
        xt[:, :plan.npc] = x_f32[base:base + plan.npc].T
        in_maps.append({
            "x_tab": x_f32,
            "xT_own": xt,
            "idx_lo": plan.idx_lo[c], "idx_hi": plan.idx_hi[c],
            "dst_lo": plan.dst_lo[c], "dst_hi": plan.dst_hi[c],
            "invc": plan.invc_tiled[c],
            "wl1": wl1, "wr1": wr1, "wl2p": wl2p, "wr2": wr2,
            "b1": b1c, "b2": b2bc, "iota": iota, "ident": ident,
        })
    return in_maps


def build_program(plan: Plan, repeats: int = 1, single_core: bool = False):
    n = plan.n_nodes
    f = plan.n_feat
    hid = plan.n_hid
    ncl = plan.n_class
    nt = plan.nt
    npc = plan.npc
    zc = P
    nl, nh = plan.nl, plan.nh
    win = plan.win
    kb = plan.kb
    f32 = mybir.dt.float32
    bf16 = mybir.dt.bfloat16

    nc = bacc.Bacc("TRN2", target_bir_lowering=False, debug=False,
                   enable_asserts=False,
                   num_devices=1 if single_core else plan.n_cores)

    x_tab = nc.dram_tensor("x_tab", [n, f], f32, kind="ExternalInput")
    xT_own = nc.dram_tensor("xT_own", [P, nt * P], f32, kind="ExternalInput")
    idx_lo = nc.dram_tensor("idx_lo", [P, nl // 16], mybir.dt.int16, kind="ExternalInput")
    idx_hi = nc.dram_tensor("idx_hi", [P, nh // 16], mybir.dt.int16, kind="ExternalInput")
    dst_lo = nc.dram_tensor("dst_lo", [P, nl // P], f32, kind="ExternalInput")
    dst_hi = nc.dram_tensor("dst_hi", [P, nh // P], f32, kind="ExternalInput")
    invc_d = nc.dram_tensor("invc", [P, nt], f32, kind="ExternalInput")
    wl1_d = nc.dram_tensor("wl1", [f, hid], f32, kind="ExternalInput")
    wr1_d = nc.dram_tensor("wr1", [f, hid], f32, kind="ExternalInput")
    wl2p_d = nc.dram_tensor("wl2p", [hid, zc], f32, kind="ExternalInput")
    wr2_d = nc.dram_tensor("wr2", [hid, ncl], f32, kind="ExternalInput")
    b1_d = nc.dram_tensor("b1", [hid, 1], f32, kind="ExternalInput")
    b2_d = nc.dram_tensor("b2", [P, ncl], f32, kind="ExternalInput")
    iota_d = nc.dram_tensor("iota", [P, P], f32, kind="ExternalInput")
    ident_d = nc.dram_tensor("ident", [P, P], f32, kind="ExternalInput")
    out_d = nc.dram_tensor("out", [npc, ncl], f32, kind="ExternalOutput")

    with tile.TileContext(nc) as tc:
        nc.gpsimd.load_library(library_config.mlp)
        with tc.tile_pool(name="const", bufs=1) as cp, \
             tc.tile_pool(name="store", bufs=1) as sp, \
             tc.tile_pool(name="msgs", bufs=3) as mp, \
             tc.tile_pool(name="oh", bufs=3) as ohp, \
             tc.tile_pool(name="fin", bufs=2) as fp, \
             tc.tile_pool(name="seg", bufs=2, space="PSUM") as psum_seg, \
             tc.tile_pool(name="paux", bufs=1, space="PSUM") as psum_aux, \
             tc.tile_pool(name="phT", bufs=2, space="PSUM") as psum_h, \
             tc.tile_pool(name="dram", bufs=1, space="DRAM") as dp:

            # ---- constant staging ----
            def load_const(dram, shape, dtype=f32, tag=""):
                t = cp.tile(shape, dtype, tag=tag)
                nc.sync.dma_start(t[:], dram[:])
                return t
            iota_t = load_const(iota_d, [P, P], tag="iota")
            ident_t = load_const(ident_d, [P, P], tag="ident")
            wl1_t = load_const(wl1_d, [f, hid], tag="wl1")
            wr1_t = load_const(wr1_d, [f, hid], tag="wr1")
            wl2p_t = load_const(wl2p_d, [hid, zc], tag="wl2p")
            wr2_t = load_const(wr2_d, [hid, ncl], tag="wr2")
            b1_t = load_const(b1_d, [hid, 1], tag="b1")
            b2_t = load_const(b2_d, [P, ncl], tag="b2")
            invc_t = load_const(invc_d, [P, nt], tag="invc")
            xT_t = load_const(xT_own, [P, nt * P], tag="xT")
            il_t = load_const(idx_lo, [P, nl // 16], mybir.dt.int16, tag="il")
            ih_t = load_const(idx_hi, [P, nh // 16], mybir.dt.int16, tag="ih")
            dl_t = load_const(dst_lo, [P, nl // P], tag="dl")
            dh_t = load_const(dst_hi, [P, nh // P], tag="dh")

            hT_store = sp.tile([P, nt * P], f32, tag="hT_store")  # [hid, node]

            z_own = dp.tile([npc, zc], bf16)
            z_full = dp.tile([n, zc], bf16, addr_space="Shared")

            # chunk schedule per stream: list of (tile_idx) per chunk
            budget = plan.budget

            def stream_schedule(g):
                sched = []
                for t in range(nt):
                    sched += [t] * int(budget[t, g])
                return sched

            sched_l = stream_schedule(0)
            sched_h = stream_schedule(1)

            for _rep in range(repeats):
                # ================= LAYER 1 =================
                def run_layer(layer):
                    spl = plan.split
                    if layer == 1:
                        tabs = (x_tab[:spl, :], x_tab[spl:, :])
                        mdt, esize = f32, f
                    else:
                        tabs = (z_full[:spl, :], z_full[spl:, :])
                        mdt, esize = bf16, zc
                    idx_tiles = (il_t, ih_t)
                    dst_tiles = (dl_t, dh_t)
                    totals = (nl, nh)
                    scheds = (sched_l, sched_h)

                    msg_bufs = {}   # (g, w) -> tile
                    oh_bufs = {}    # (g, j) -> tile

                    def ensure_win(g, w):
                        key = (g, w)
                        if key in msg_bufs:
                            return msg_bufs[key]
                        lo = w * win
                        cnt = min(win, totals[g] - lo)
                        mt = mp.tile([P, win // P, esize], mdt, tag="msgs")
                        nc.gpsimd.dma_gather(
                            mt[:, :cnt // P, :], tabs[g], idx_tiles[g][:, lo // 16:(lo + cnt) // 16],
                            cnt, cnt, esize)
                        msg_bufs[key] = mt
                        return mt

                    def ensure_oh(g, j):
                        key = (g, j)
                        if key in oh_bufs:
                            return oh_bufs[key]
                        lo = j * kb
                        ncols = min(kb, totals[g] // P - lo)
                        t = ohp.tile([P, kb, P], mdt, tag="oh")
                        dst_sl = dst_tiles[g][:, lo:lo + ncols, None].to_broadcast((P, ncols, P))
                        iota_b = iota_t[:, None, :].to_broadcast((P, ncols, P))
                        nc.vector.tensor_tensor(out=t[:, :ncols, :], in0=dst_sl, in1=iota_b,
                                                op=mybir.AluOpType.is_equal)
                        oh_bufs[key] = t
                        return t

                    chunk_pos = [0, 0]
                    for t in range(nt):
                        pt = psum_seg.tile([P, esize if layer == 2 else f], f32, tag="seg")
                        first = True
                        nchunks = int(budget[t, 0]) + int(budget[t, 1])
                        done = 0
                        for g in (0, 1):
                            for _ in range(int(budget[t, g])):
                                ci = chunk_pos[g]
                                chunk_pos[g] += 1
                                done += 1
                                w, col = divmod(ci * P, win)
                                mt = ensure_win(g, w)
                                oh = ensure_oh(g, ci // kb)
                                if layer == 1:
                                    rhs = mt[:, col // P, :]
                                else:
                                    rhs = mt[:, col // P, 0:64]
                                nc.tensor.matmul(
                                    out=pt[:, 0:64] if layer == 2 else pt[:],
                                    lhsT=oh[:, ci % kb, :], rhs=rhs,
                                    start=first, stop=(done == nchunks))
                                first = False
                        rows = min(P, npc - t * P)
                        if layer == 1:
                            # mean-scale, transpose, dense matmuls, relu
                            aggm = fp.tile([P, f], f32, tag="aggm")
                            if nchunks == 0:
                                nc.vector.memset(aggm[:], 0.0)
                            else:
                                nc.vector.tensor_scalar(
                                    out=aggm[:], in0=pt[:, :f], scalar1=invc_t[:, t:t + 1],
                                    scalar2=None, op0=mybir.AluOpType.mult)
                            paggT = psum_aux.tile([P, P], f32, tag="aggT")
                            nc.tensor.transpose(out=paggT[:], in_=aggm[:], identity=ident_t[:])
                            aggT = fp.tile([P, P], f32, tag="aggT_sb")
                            nc.vector.tensor_copy(out=aggT[:], in_=paggT[:])
                            phT = psum_h.tile([P, P], f32, tag="hT")
                            nc.tensor.matmul(out=phT[:], lhsT=wl1_t[:], rhs=aggT[:],
                                             start=True, stop=False)
                            nc.tensor.matmul(out=phT[:], lhsT=wr1_t[:],
                                             rhs=xT_t[:, t * P:(t + 1) * P],
                                             start=False, stop=True)
                            hT_sl = hT_store[:, t * P:(t + 1) * P]
                            nc.scalar.activation(out=hT_sl, in_=phT[:],
                                                 func=mybir.ActivationFunctionType.Relu,
                                                 bias=b1_t[:], scale=1.0)
                            pz = psum_aux.tile([P, zc], f32, tag="z")
                            nc.tensor.matmul(out=pz[:], lhsT=hT_sl, rhs=wl2p_t[:],
                                             start=True, stop=True)
                            zsb = fp.tile([P, zc], bf16, tag="zsb")
                            nc.vector.tensor_copy(out=zsb[:], in_=pz[:])
                            nc.sync.dma_start(z_own[t * P:t * P + rows, :], zsb[:rows, :])
                        else:
                            s2 = fp.tile([P, ncl], f32, tag="s2")
                            if nchunks == 0:
                                nc.vector.memset(s2[:], 0.0)
                            else:
                                nc.vector.tensor_scalar(
                                    out=s2[:], in0=pt[:, 0:ncl], scalar1=invc_t[:, t:t + 1],
                                    scalar2=None, op0=mybir.AluOpType.mult)
                            po = psum_aux.tile([P, P], f32, tag="aggT")
                            nc.tensor.matmul(out=po[:, 0:ncl], lhsT=hT_store[:, t * P:(t + 1) * P],
                                             rhs=wr2_t[:], start=True, stop=True)
                            ofin = fp.tile([P, ncl], f32, tag="ofin")
                            nc.vector.tensor_add(out=ofin[:], in0=po[:, 0:ncl], in1=s2[:])
                            nc.vector.tensor_add(out=ofin[:], in0=ofin[:], in1=b2_t[:, :ncl])
                            nc.sync.dma_start(out_d[t * P:t * P + rows, :], ofin[:rows, :])

                run_layer(1)
                nc.gpsimd.collective_compute(
                    "AllGather", mybir.AluOpType.bypass,
                    replica_groups=[list(range(plan.n_cores))],
                    ins=[z_own[:]], outs=[z_full[:]])
                run_layer(2)

    nc.compile()
    return nc


import numpy as np
import jax
from jax.sharding import Mesh, PartitionSpec
from jax.experimental.shard_map import shard_map
import concourse.mybir as mybir
import concourse.bass2jax as bass2jax
from concourse.bass2jax import _bass_exec_p, partition_id_tensor, install_neuronx_cc_hook


class SpmdRunner:
    def __init__(self, nc, n_cores: int):
        install_neuronx_cc_hook()
        self.nc = nc
        self.n_cores = n_cores
        partition_name = nc.partition_id_tensor.name if nc.partition_id_tensor else None
        in_names, out_names, out_avals = [], [], []
        zero_outs = []
        for alloc in nc.m.functions[0].allocations:
            if not isinstance(alloc, mybir.MemoryLocationSet):
                continue
            name = alloc.memorylocations[0].name
            if alloc.kind == "ExternalInput":
                if name != partition_name:
                    in_names.append(name)
            elif alloc.kind == "ExternalOutput":
                shape = tuple(alloc.tensor_shape)
                dtype = mybir.dt.np(alloc.dtype)
                out_names.append(name)
                out_avals.append(jax.core.ShapedArray(shape, dtype))
                zero_outs.append(np.zeros(shape, dtype))
        self.in_names = list(in_names)
        self.out_names = out_names
        self.out_avals = out_avals
        self.zero_outs = zero_outs
        n_params = len(in_names)
        all_in_names = list(in_names) + list(out_names)
        if partition_name is not None:
            all_in_names.append(partition_name)

        def _body(*args):
            operands = list(args)
            if partition_name is not None:
                operands.append(partition_id_tensor())
            outs = _bass_exec_p.bind(
                *operands,
                out_avals=tuple(out_avals),
                in_names=tuple(all_in_names),
                out_names=tuple(out_names),
                lowering_input_output_aliases=(),
                sim_require_finite=False,
                sim_require_nnan=False,
                nc=nc,
            )
            return tuple(outs)

        devices = jax.devices()[:n_cores]
        assert len(devices) == n_cores
        self.mesh = Mesh(np.asarray(devices), ("core",))
        in_specs = (PartitionSpec("core"),) * (n_params + len(out_names))
        out_specs = (PartitionSpec("core"),) * len(out_names)
        self.fn = jax.jit(
            shard_map(_body, mesh=self.mesh, in_specs=in_specs,
                      out_specs=out_specs, check_rep=False),
            keep_unused=True,
        )
        self._dev_args = None

    def stage(self, in_maps):
        """Concatenate per-core inputs and device_put once."""
        n = self.n_cores
        concat_in = [
            np.concatenate([np.asarray(in_maps[c][name]) for c in range(n)], axis=0)
            for name in self.in_names
        ]
        concat_zeros = [
            np.zeros((n * z.shape[0], *z.shape[1:]), z.dtype) for z in self.zero_outs
        ]
        from jax.sharding import NamedSharding
        sh = NamedSharding(self.mesh, PartitionSpec("core"))
        self._dev_args = [jax.device_put(a, sh) for a in concat_in + concat_zeros]
        return self

    def run(self):
        return self.fn(*self._dev_args)

    def run_blocking(self):
        out = self.fn(*self._dev_args)
        jax.block_until_ready(out)
        return out

    def results(self, out_arrs):
        n = self.n_cores
        return [
            {name: np.asarray(out_arrs[i]).reshape(n, *self.out_avals[i].shape)[c]
             for i, name in enumerate(self.out_names)}
            for c in range(n)
        ]


# ---------------- self-contained entry point ----------------
_CACHE = {}

def kernel(**inputs):
    import numpy as _np
    x = _np.asarray(inputs["x"], dtype=_np.float32)
    edge_index = _np.asarray(inputs["edge_index"])
    Wl1 = _np.asarray(inputs["Wl1"], dtype=_np.float32)
    Wr1 = _np.asarray(inputs["Wr1"], dtype=_np.float32)
    b1 = _np.asarray(inputs["b1"], dtype=_np.float32)
    Wl2 = _np.asarray(inputs["Wl2"], dtype=_np.float32)
    Wr2 = _np.asarray(inputs["Wr2"], dtype=_np.float32)
    b2 = _np.asarray(inputs["b2"], dtype=_np.float32)
    N, F = x.shape
    H = Wl1.shape[1]
    C = Wl2.shape[1]
    import hashlib
    eh = hashlib.md5(edge_index.tobytes()).hexdigest()
    key = ("plan", N, F, H, C, edge_index.shape[1], eh)
    if key not in _CACHE:
        plan = make_plan(edge_index, N, F, H, C, 8, win=1024, kb=8)
        nc = build_program(plan)
        runner = SpmdRunner(nc, 8)
        _CACHE[key] = (plan, runner)
    plan, runner = _CACHE[key]
    in_maps = stage_inputs(plan, x, Wl1, Wr1, b1, Wl2, Wr2, b2)
    runner.stage(in_maps)
    out_arrs = runner.run_blocking()
    results = runner.results(out_arrs)
    out = _np.concatenate([results[c]["out"] for c in range(8)], axis=0)
    return out[:N].astype(_np.float32)



# revision 3
# speedup vs baseline: 41.0795x; 41.0795x over previous
"""GraphSAGE 2-layer GNN on TRN2, 8-core SPMD Bass/Tile kernel.

Strategy:
- Nodes sharded across 8 cores (6250 each). Edges partitioned by dst core.
- x replicated in each core's HBM; per-edge neighbor features fetched with
  dma_gather (512B rows, ~HBM line rate).
- Segment-sum via one-hot matmul on PE: edges sorted by dst tile; for each
  128-edge chunk, onehot[e,d] = (dstloc[e]==d) built on DVE, then
  psum[dst,feat] += onehot.T @ msgs.
- int16 gather indices => split edges into low (src<32768) / high streams.
- Layer 2 gathers z = h @ Wl2 (padded to 128 bf16 cols = 256B rows) after an
  8-rank AllGather of per-core z slices.
"""
from dataclasses import dataclass, field
import numpy as np
import ml_dtypes

import concourse.bacc as bacc
import concourse.bass as bass
import concourse.mybir as mybir
import concourse.tile as tile
from concourse import library_config

P = 128


@dataclass
class Plan:
    n_nodes: int
    n_feat: int
    n_hid: int
    n_class: int
    n_cores: int
    npc: int                 # nodes per core
    nt: int                  # dst tiles per core
    win: int                 # gather window (slots per dma_gather call)
    kb: int                  # chunks per one-hot build
    split: int               # low/high src index split (int16 limit)
    budget: np.ndarray       # [nt, 2] chunks per (tile, group)
    nl: int = 0              # total slots, low stream
    nh: int = 0              # total slots, high stream
    # per-core staged arrays
    idx_lo: list = field(default_factory=list)   # [128, nl/16] int16
    idx_hi: list = field(default_factory=list)
    dst_lo: list = field(default_factory=list)   # [128, nl/128] f32
    dst_hi: list = field(default_factory=list)
    invc_tiled: list = field(default_factory=list)  # [128, nt] f32
    xT_own: list = field(default_factory=list)      # [128, nt*128] f32


def _wrap_idx(arr_i16: np.ndarray) -> np.ndarray:
    # position j -> partition j%16, col j//16; replicated 8x down partitions
    w = arr_i16.reshape(-1, 16).T            # [16, n/16]
    return np.ascontiguousarray(np.tile(w, (8, 1)))  # [128, n/16]


def _wrap_slots(arr_f32: np.ndarray) -> np.ndarray:
    # position j -> partition j%128, col j//128 (matches dma_gather output)
    return np.ascontiguousarray(arr_f32.reshape(-1, P).T)  # [128, n/128]


def make_plan(edge_index: np.ndarray, n_nodes: int, n_feat: int, n_hid: int,
              n_class: int, n_cores: int, win: int = 1024, kb: int = 8,
              split: int = 32768) -> Plan:
    src = np.asarray(edge_index[0], dtype=np.int64)
    dst = np.asarray(edge_index[1], dtype=np.int64)
    npc = n_nodes // n_cores
    assert npc * n_cores == n_nodes
    nt = (npc + P - 1) // P

    deg = np.bincount(dst, minlength=n_nodes).astype(np.float64)
    invc = (1.0 / np.maximum(deg, 1.0)).astype(np.float32)

    core_of = dst // npc
    tloc = (dst - core_of * npc) // P          # 0..nt-1
    grp = (src >= split).astype(np.int64)      # 0=low, 1=high

    # counts[c, t, g]
    counts = np.zeros((n_cores, nt, 2), dtype=np.int64)
    np.add.at(counts, (core_of, tloc, grp), 1)
    budget = np.ceil(counts.max(axis=0) / P).astype(np.int64)  # [nt, 2] chunks

    plan = Plan(n_nodes=n_nodes, n_feat=n_feat, n_hid=n_hid, n_class=n_class,
                n_cores=n_cores, npc=npc, nt=nt, win=win, kb=kb, split=split,
                budget=budget)
    nl = int(budget[:, 0].sum()) * P
    nh = int(budget[:, 1].sum()) * P
    plan.nl, plan.nh = nl, nh

    # slot offsets per (t, g) within each stream
    off_l = np.concatenate([[0], np.cumsum(budget[:, 0])])[:-1] * P
    off_h = np.concatenate([[0], np.cumsum(budget[:, 1])])[:-1] * P
    plan.off_l, plan.off_h = off_l, off_h

    order = np.argsort(core_of * (nt * 2) + tloc * 2 + grp, kind="stable")
    for c in range(n_cores):
        idxs = {0: np.zeros(nl, np.int16), 1: np.zeros(nh, np.int16)}
        dsts = {0: np.full(nl, -1.0, np.float32), 1: np.full(nh, -1.0, np.float32)}
        offs = {0: off_l, 1: off_h}
        sel = order[np.searchsorted(core_of[order], c, side="left"):
                    np.searchsorted(core_of[order], c, side="right")]
        # sel is sorted by (t, g); group contiguous runs
        st = src[sel]
        dt_ = dst[sel]
        tl = tloc[sel]
        gl = grp[sel]
        key = tl * 2 + gl
        boundaries = np.concatenate([[0], np.where(np.diff(key) != 0)[0] + 1, [len(sel)]])
        for b0, b1 in zip(boundaries[:-1], boundaries[1:]):
            t = int(tl[b0]); g = int(gl[b0])
            n = b1 - b0
            o = int(offs[g][t])
            s_ids = st[b0:b1]
            idxs[g][o:o + n] = (s_ids - (split if g else 0)).astype(np.int16)
            dsts[g][o:o + n] = (dt_[b0:b1] - c * npc - t * P).astype(np.float32)
        plan.idx_lo.append(_wrap_idx(idxs[0]))
        plan.idx_hi.append(_wrap_idx(idxs[1]))
        plan.dst_lo.append(_wrap_slots(dsts[0]))
        plan.dst_hi.append(_wrap_slots(dsts[1]))
        ic = np.zeros((P, nt), np.float32)
        base = c * npc
        n_own = npc
        icl = invc[base:base + n_own]
        icl = np.concatenate([icl, np.zeros(nt * P - n_own, np.float32)])
        ic[:, :] = icl.reshape(nt, P).T
        plan.invc_tiled.append(np.ascontiguousarray(ic))
    return plan


def stage_inputs(plan: Plan, x, Wl1, Wr1, b1, Wl2, Wr2, b2):
    """Build per-core in_maps (numpy) for the bass program."""
    n, f = x.shape
    hid = plan.n_hid
    ncl = plan.n_class
    zcols = P  # bf16 z row padded to 128 cols = 256B
    x_f32 = np.ascontiguousarray(np.asarray(x, dtype=np.float32))
    wl1 = np.ascontiguousarray(np.asarray(Wl1, np.float32))
    wr1 = np.ascontiguousarray(np.asarray(Wr1, np.float32))
    wl2p = np.zeros((hid, zcols), np.float32)
    wl2p[:, :ncl] = np.asarray(Wl2, np.float32)
    wr2 = np.ascontiguousarray(np.asarray(Wr2, np.float32))
    b1c = np.asarray(b1, np.float32).reshape(hid, 1)
    b2bc = np.broadcast_to(np.asarray(b2, np.float32), (P, ncl)).copy()
    iota = np.broadcast_to(np.arange(P, dtype=np.float32), (P, P)).copy()
    ident = np.eye(P, dtype=np.float32)

    in_maps = []
    for c in range(plan.n_cores):
        base = c * plan.npc
        xt = np.zeros((P, plan.nt * P), np.float32)
        xt[:, :plan.npc] = x_f32[base:base + plan.npc].T
        in_maps.append({
            "x_tab": x_f32,
            "xT_own": xt,
            "idx_lo": plan.idx_lo[c], "idx_hi": plan.idx_hi[c],
            "dst_lo": plan.dst_lo[c], "dst_hi": plan.dst_hi[c],
            "invc": plan.invc_tiled[c],
            "wl1": wl1, "wr1": wr1, "wl2p": wl2p, "wr2": wr2,
            "b1": b1c, "b2": b2bc, "iota": iota, "ident": ident,
        })
    return in_maps


def build_program(plan: Plan, repeats: int = 1, single_core: bool = False):
    n = plan.n_nodes
    f = plan.n_feat
    hid = plan.n_hid
    ncl = plan.n_class
    nt = plan.nt
    npc = plan.npc
    zc = P
    nl, nh = plan.nl, plan.nh
    win = plan.win
    kb = plan.kb
    f32 = mybir.dt.float32
    bf16 = mybir.dt.bfloat16

    nc = bacc.Bacc("TRN2", target_bir_lowering=False, debug=False,
                   enable_asserts=False,
                   num_devices=1 if single_core else plan.n_cores)

    x_tab = nc.dram_tensor("x_tab", [n, f], f32, kind="ExternalInput")
    xT_own = nc.dram_tensor("xT_own", [P, nt * P], f32, kind="ExternalInput")
    idx_lo = nc.dram_tensor("idx_lo", [P, nl // 16], mybir.dt.int16, kind="ExternalInput")
    idx_hi = nc.dram_tensor("idx_hi", [P, nh // 16], mybir.dt.int16, kind="ExternalInput")
    dst_lo = nc.dram_tensor("dst_lo", [P, nl // P], f32, kind="ExternalInput")
    dst_hi = nc.dram_tensor("dst_hi", [P, nh // P], f32, kind="ExternalInput")
    invc_d = nc.dram_tensor("invc", [P, nt], f32, kind="ExternalInput")
    wl1_d = nc.dram_tensor("wl1", [f, hid], f32, kind="ExternalInput")
    wr1_d = nc.dram_tensor("wr1", [f, hid], f32, kind="ExternalInput")
    wl2p_d = nc.dram_tensor("wl2p", [hid, zc], f32, kind="ExternalInput")
    wr2_d = nc.dram_tensor("wr2", [hid, ncl], f32, kind="ExternalInput")
    b1_d = nc.dram_tensor("b1", [hid, 1], f32, kind="ExternalInput")
    b2_d = nc.dram_tensor("b2", [P, ncl], f32, kind="ExternalInput")
    iota_d = nc.dram_tensor("iota", [P, P], f32, kind="ExternalInput")
    ident_d = nc.dram_tensor("ident", [P, P], f32, kind="ExternalInput")
    out_d = nc.dram_tensor("out", [npc, ncl], f32, kind="ExternalOutput")

    with tile.TileContext(nc) as tc:
        nc.gpsimd.load_library(library_config.mlp)
        with tc.tile_pool(name="const", bufs=1) as cp, \
             tc.tile_pool(name="store", bufs=1) as sp, \
             tc.tile_pool(name="msgs", bufs=3) as mp, \
             tc.tile_pool(name="oh", bufs=3) as ohp, \
             tc.tile_pool(name="fin", bufs=2) as fp, \
             tc.tile_pool(name="seg", bufs=2, space="PSUM") as psum_seg, \
             tc.tile_pool(name="paux", bufs=1, space="PSUM") as psum_aux, \
             tc.tile_pool(name="phT", bufs=2, space="PSUM") as psum_h, \
             tc.tile_pool(name="dram", bufs=1, space="DRAM") as dp:

            # ---- constant staging ----
            def load_const(dram, shape, dtype=f32, tag=""):
                t = cp.tile(shape, dtype, tag=tag)
                nc.sync.dma_start(t[:], dram[:])
                return t
            iota_t = load_const(iota_d, [P, P], tag="iota")
            ident_t = load_const(ident_d, [P, P], tag="ident")
            wl1_t = load_const(wl1_d, [f, hid], tag="wl1")
            wr1_t = load_const(wr1_d, [f, hid], tag="wr1")
            wl2p_t = load_const(wl2p_d, [hid, zc], tag="wl2p")
            wr2_t = load_const(wr2_d, [hid, ncl], tag="wr2")
            b1_t = load_const(b1_d, [hid, 1], tag="b1")
            b2_t = load_const(b2_d, [P, ncl], tag="b2")
            invc_t = load_const(invc_d, [P, nt], tag="invc")
            xT_t = load_const(xT_own, [P, nt * P], tag="xT")
            il_t = load_const(idx_lo, [P, nl // 16], mybir.dt.int16, tag="il")
            ih_t = load_const(idx_hi, [P, nh // 16], mybir.dt.int16, tag="ih")
            dl_t = load_const(dst_lo, [P, nl // P], tag="dl")
            dh_t = load_const(dst_hi, [P, nh // P], tag="dh")

            hT_store = sp.tile([P, nt * P], f32, tag="hT_store")  # [hid, node]

            # chunk schedule per stream: list of (tile_idx) per chunk
            budget = plan.budget

            def stream_schedule(g):
                sched = []
                for t in range(nt):
                    sched += [t] * int(budget[t, g])
                return sched

            sched_l = stream_schedule(0)
            sched_h = stream_schedule(1)

            for _rep in range(repeats):
                z_own = dp.tile([npc, zc], bf16, tag=f"z_own{_rep}")
                z_full = dp.tile([n, zc], bf16, addr_space="Shared", tag=f"z_full{_rep}")

                # ================= LAYER 1 =================
                def run_layer(layer):
                    spl = plan.split
                    if layer == 1:
                        tabs = (x_tab[:spl, :], x_tab[spl:, :])
                        mdt, esize = f32, f
                    else:
                        tabs = (z_full[:spl, :], z_full[spl:, :])
                        mdt, esize = bf16, zc
                    idx_tiles = (il_t, ih_t)
                    dst_tiles = (dl_t, dh_t)
                    totals = (nl, nh)
                    scheds = (sched_l, sched_h)

                    msg_bufs = {}   # (g, w) -> tile
                    oh_bufs = {}    # (g, j) -> tile

                    def ensure_win(g, w):
                        key = (g, w)
                        if key in msg_bufs:
                            return msg_bufs[key]
                        lo = w * win
                        cnt = min(win, totals[g] - lo)
                        mt = mp.tile([P, win // P, esize], mdt, tag="msgs")
                        nc.gpsimd.dma_gather(
                            mt[:, :cnt // P, :], tabs[g], idx_tiles[g][:, lo // 16:(lo + cnt) // 16],
                            cnt, cnt, esize)
                        msg_bufs[key] = mt
                        return mt

                    def ensure_oh(g, j):
                        key = (g, j)
                        if key in oh_bufs:
                            return oh_bufs[key]
                        lo = j * kb
                        ncols = min(kb, totals[g] // P - lo)
                        t = ohp.tile([P, kb, P], mdt, tag="oh")
                        dst_sl = dst_tiles[g][:, lo:lo + ncols, None].to_broadcast((P, ncols, P))
                        iota_b = iota_t[:, None, :].to_broadcast((P, ncols, P))
                        nc.vector.tensor_tensor(out=t[:, :ncols, :], in0=dst_sl, in1=iota_b,
                                                op=mybir.AluOpType.is_equal)
                        oh_bufs[key] = t
                        return t

                    chunk_pos = [0, 0]
                    for t in range(nt):
                        pt = psum_seg.tile([P, esize if layer == 2 else f], f32, tag="seg")
                        first = True
                        nchunks = int(budget[t, 0]) + int(budget[t, 1])
                        done = 0
                        for g in (0, 1):
                            for _ in range(int(budget[t, g])):
                                ci = chunk_pos[g]
                                chunk_pos[g] += 1
                                done += 1
                                w, col = divmod(ci * P, win)
                                mt = ensure_win(g, w)
                                oh = ensure_oh(g, ci // kb)
                                if layer == 1:
                                    rhs = mt[:, col // P, :]
                                else:
                                    rhs = mt[:, col // P, 0:64]
                                nc.tensor.matmul(
                                    out=pt[:, 0:64] if layer == 2 else pt[:],
                                    lhsT=oh[:, ci % kb, :], rhs=rhs,
                                    start=first, stop=(done == nchunks))
                                first = False
                        rows = min(P, npc - t * P)
                        if layer == 1:
                            # mean-scale, transpose, dense matmuls, relu
                            aggm = fp.tile([P, f], f32, tag="aggm")
                            if nchunks == 0:
                                nc.vector.memset(aggm[:], 0.0)
                            else:
                                nc.vector.tensor_scalar(
                                    out=aggm[:], in0=pt[:, :f], scalar1=invc_t[:, t:t + 1],
                                    scalar2=None, op0=mybir.AluOpType.mult)
                            paggT = psum_aux.tile([P, P], f32, tag="aggT")
                            nc.tensor.transpose(out=paggT[:], in_=aggm[:], identity=ident_t[:])
                            aggT = fp.tile([P, P], f32, tag="aggT_sb")
                            nc.vector.tensor_copy(out=aggT[:], in_=paggT[:])
                            phT = psum_h.tile([P, P], f32, tag="hT")
                            nc.tensor.matmul(out=phT[:], lhsT=wl1_t[:], rhs=aggT[:],
                                             start=True, stop=False)
                            nc.tensor.matmul(out=phT[:], lhsT=wr1_t[:],
                                             rhs=xT_t[:, t * P:(t + 1) * P],
                                             start=False, stop=True)
                            hT_sl = hT_store[:, t * P:(t + 1) * P]
                            nc.scalar.activation(out=hT_sl, in_=phT[:],
                                                 func=mybir.ActivationFunctionType.Relu,
                                                 bias=b1_t[:], scale=1.0)
                            pz = psum_aux.tile([P, zc], f32, tag="z")
                            nc.tensor.matmul(out=pz[:], lhsT=hT_sl, rhs=wl2p_t[:],
                                             start=True, stop=True)
                            zsb = fp.tile([P, zc], bf16, tag="zsb")
                            nc.vector.tensor_copy(out=zsb[:], in_=pz[:])
                            nc.sync.dma_start(z_own[t * P:t * P + rows, :], zsb[:rows, :])
                        else:
                            s2 = fp.tile([P, ncl], f32, tag="s2")
                            if nchunks == 0:
                                nc.vector.memset(s2[:], 0.0)
                            else:
                                nc.vector.tensor_scalar(
                                    out=s2[:], in0=pt[:, 0:ncl], scalar1=invc_t[:, t:t + 1],
                                    scalar2=None, op0=mybir.AluOpType.mult)
                            po = psum_aux.tile([P, P], f32, tag="aggT")
                            nc.tensor.matmul(out=po[:, 0:ncl], lhsT=hT_store[:, t * P:(t + 1) * P],
                                             rhs=wr2_t[:], start=True, stop=True)
                            ofin = fp.tile([P, ncl], f32, tag="ofin")
                            nc.vector.tensor_add(out=ofin[:], in0=po[:, 0:ncl], in1=s2[:])
                            nc.vector.tensor_add(out=ofin[:], in0=ofin[:], in1=b2_t[:, :ncl])
                            nc.sync.dma_start(out_d[t * P:t * P + rows, :], ofin[:rows, :])

                run_layer(1)
                nc.gpsimd.collective_compute(
                    "AllGather", mybir.AluOpType.bypass,
                    replica_groups=[list(range(plan.n_cores))],
                    ins=[z_own[:]], outs=[z_full[:]])
                run_layer(2)

    nc.compile()
    return nc


import numpy as np
import jax
from jax.sharding import Mesh, PartitionSpec
from jax.experimental.shard_map import shard_map
import concourse.mybir as mybir
import concourse.bass2jax as bass2jax
from concourse.bass2jax import _bass_exec_p, partition_id_tensor, install_neuronx_cc_hook


class SpmdRunner:
    def __init__(self, nc, n_cores: int):
        install_neuronx_cc_hook()
        self.nc = nc
        self.n_cores = n_cores
        partition_name = nc.partition_id_tensor.name if nc.partition_id_tensor else None
        in_names, out_names, out_avals = [], [], []
        zero_outs = []
        for alloc in nc.m.functions[0].allocations:
            if not isinstance(alloc, mybir.MemoryLocationSet):
                continue
            name = alloc.memorylocations[0].name
            if alloc.kind == "ExternalInput":
                if name != partition_name:
                    in_names.append(name)
            elif alloc.kind == "ExternalOutput":
                shape = tuple(alloc.tensor_shape)
                dtype = mybir.dt.np(alloc.dtype)
                out_names.append(name)
                out_avals.append(jax.core.ShapedArray(shape, dtype))
                zero_outs.append(np.zeros(shape, dtype))
        self.in_names = list(in_names)
        self.out_names = out_names
        self.out_avals = out_avals
        self.zero_outs = zero_outs
        n_params = len(in_names)
        all_in_names = list(in_names) + list(out_names)
        if partition_name is not None:
            all_in_names.append(partition_name)

        def _body(*args):
            operands = list(args)
            if partition_name is not None:
                operands.append(partition_id_tensor())
            outs = _bass_exec_p.bind(
                *operands,
                out_avals=tuple(out_avals),
                in_names=tuple(all_in_names),
                out_names=tuple(out_names),
                lowering_input_output_aliases=(),
                sim_require_finite=False,
                sim_require_nnan=False,
                nc=nc,
            )
            return tuple(outs)

        devices = jax.devices()[:n_cores]
        assert len(devices) == n_cores
        self.mesh = Mesh(np.asarray(devices), ("core",))
        in_specs = (PartitionSpec("core"),) * (n_params + len(out_names))
        out_specs = (PartitionSpec("core"),) * len(out_names)
        self.fn = jax.jit(
            shard_map(_body, mesh=self.mesh, in_specs=in_specs,
                      out_specs=out_specs, check_rep=False),
            keep_unused=True,
        )
        self._dev_args = None

    def stage(self, in_maps):
        """Concatenate per-core inputs and device_put once."""
        n = self.n_cores
        concat_in = [
            np.concatenate([np.asarray(in_maps[c][name]) for c in range(n)], axis=0)
            for name in self.in_names
        ]
        concat_zeros = [
            np.zeros((n * z.shape[0], *z.shape[1:]), z.dtype) for z in self.zero_outs
        ]
        from jax.sharding import NamedSharding
        sh = NamedSharding(self.mesh, PartitionSpec("core"))
        self._dev_args = [jax.device_put(a, sh) for a in concat_in + concat_zeros]
        return self

    def run(self):
        return self.fn(*self._dev_args)

    def run_blocking(self):
        out = self.fn(*self._dev_args)
        jax.block_until_ready(out)
        return out

    def results(self, out_arrs):
        n = self.n_cores
        return [
            {name: np.asarray(out_arrs[i]).reshape(n, *self.out_avals[i].shape)[c]
             for i, name in enumerate(self.out_names)}
            for c in range(n)
        ]


# ---------------- self-contained entry point ----------------
_CACHE = {}

def kernel(**inputs):
    import numpy as _np
    x = _np.asarray(inputs["x"], dtype=_np.float32)
    edge_index = _np.asarray(inputs["edge_index"])
    Wl1 = _np.asarray(inputs["Wl1"], dtype=_np.float32)
    Wr1 = _np.asarray(inputs["Wr1"], dtype=_np.float32)
    b1 = _np.asarray(inputs["b1"], dtype=_np.float32)
    Wl2 = _np.asarray(inputs["Wl2"], dtype=_np.float32)
    Wr2 = _np.asarray(inputs["Wr2"], dtype=_np.float32)
    b2 = _np.asarray(inputs["b2"], dtype=_np.float32)
    N, F = x.shape
    H = Wl1.shape[1]
    C = Wl2.shape[1]
    import hashlib
    eh = hashlib.md5(edge_index.tobytes()).hexdigest()
    key = ("plan", N, F, H, C, edge_index.shape[1], eh)
    if key not in _CACHE:
        plan = make_plan(edge_index, N, F, H, C, 8, win=1024, kb=8)
        nc = build_program(plan)
        runner = SpmdRunner(nc, 8)
        _CACHE[key] = (plan, runner)
    plan, runner = _CACHE[key]
    in_maps = stage_inputs(plan, x, Wl1, Wr1, b1, Wl2, Wr2, b2)
    runner.stage(in_maps)
    out_arrs = runner.run_blocking()
    results = runner.results(out_arrs)
    out = _np.concatenate([results[c]["out"] for c in range(8)], axis=0)
    return out[:N].astype(_np.float32)



# revision 27
# speedup vs baseline: 42.8645x; 1.0435x over previous
"""GraphSAGE 2-layer GNN on TRN2, 8-core SPMD Bass/Tile kernel.

Strategy:
- Nodes sharded across 8 cores (6250 each). Edges partitioned by dst core.
- x replicated in each core's HBM; per-edge neighbor features fetched with
  dma_gather (512B rows, ~HBM line rate).
- Segment-sum via one-hot matmul on PE: edges sorted by dst tile; for each
  128-edge chunk, onehot[e,d] = (dstloc[e]==d) built on DVE, then
  psum[dst,feat] += onehot.T @ msgs.
- int16 gather indices => split edges into low (src<32768) / high streams.
- Layer 2 gathers z = h @ Wl2 (padded to 128 bf16 cols = 256B rows) after an
  8-rank AllGather of per-core z slices.
"""
from dataclasses import dataclass, field
import numpy as np
import ml_dtypes

import concourse.bacc as bacc
import concourse.bass as bass
import concourse.mybir as mybir
import concourse.tile as tile
from concourse import library_config

P = 128


@dataclass
class Plan:
    n_nodes: int
    n_feat: int
    n_hid: int
    n_class: int
    n_cores: int
    npc: int                 # nodes per core
    nt: int                  # dst tiles per core
    win: int                 # gather window (slots per dma_gather call)
    kb: int                  # chunks per one-hot build
    split: int               # low/high src index split (int16 limit)
    budget: np.ndarray       # [nt, 2] chunks per (tile, group)
    nl: int = 0              # total slots, low stream
    nh: int = 0              # total slots, high stream
    # per-core staged arrays
    idx_lo: list = field(default_factory=list)   # [128, nl/16] int16
    idx_hi: list = field(default_factory=list)
    dst_lo: list = field(default_factory=list)   # [128, nl/128] f32
    dst_hi: list = field(default_factory=list)
    invc_tiled: list = field(default_factory=list)  # [128, nt] f32
    xT_own: list = field(default_factory=list)      # [128, nt*128] f32


def _wrap_idx(arr_i16: np.ndarray) -> np.ndarray:
    # position j -> partition j%16, col j//16; replicated 8x down partitions
    w = arr_i16.reshape(-1, 16).T            # [16, n/16]
    return np.ascontiguousarray(np.tile(w, (8, 1)))  # [128, n/16]


def _wrap_slots(arr_f32: np.ndarray) -> np.ndarray:
    # position j -> partition j%128, col j//128 (matches dma_gather output)
    return np.ascontiguousarray(arr_f32.reshape(-1, P).T)  # [128, n/128]


def make_plan(edge_index: np.ndarray, n_nodes: int, n_feat: int, n_hid: int,
              n_class: int, n_cores: int, win: int = 1024, kb: int = 16,
              split: int = 32768) -> Plan:
    src = np.asarray(edge_index[0], dtype=np.int64)
    dst = np.asarray(edge_index[1], dtype=np.int64)
    npc = n_nodes // n_cores
    assert npc * n_cores == n_nodes
    nt = (npc + P - 1) // P

    deg = np.bincount(dst, minlength=n_nodes).astype(np.float64)
    invc = (1.0 / np.maximum(deg, 1.0)).astype(np.float32)

    core_of = dst // npc
    tloc = (dst - core_of * npc) // P          # 0..nt-1
    grp = (src >= split).astype(np.int64)      # 0=low, 1=high

    # counts[c, t, g]
    counts = np.zeros((n_cores, nt, 2), dtype=np.int64)
    np.add.at(counts, (core_of, tloc, grp), 1)
    budget = np.ceil(counts.max(axis=0) / P).astype(np.int64)  # [nt, 2] chunks

    plan = Plan(n_nodes=n_nodes, n_feat=n_feat, n_hid=n_hid, n_class=n_class,
                n_cores=n_cores, npc=npc, nt=nt, win=win, kb=kb, split=split,
                budget=budget)
    nl = int(budget[:, 0].sum()) * P
    nh = int(budget[:, 1].sum()) * P
    plan.nl, plan.nh = nl, nh

    # slot offsets per (t, g) within each stream
    off_l = np.concatenate([[0], np.cumsum(budget[:, 0])])[:-1] * P
    off_h = np.concatenate([[0], np.cumsum(budget[:, 1])])[:-1] * P
    plan.off_l, plan.off_h = off_l, off_h

    order = np.argsort(core_of * (nt * 2) + tloc * 2 + grp, kind="stable")
    for c in range(n_cores):
        idxs = {0: np.zeros(nl, np.int16), 1: np.zeros(nh, np.int16)}
        dsts = {0: np.full(nl, -1.0, np.float32), 1: np.full(nh, -1.0, np.float32)}
        offs = {0: off_l, 1: off_h}
        sel = order[np.searchsorted(core_of[order], c, side="left"):
                    np.searchsorted(core_of[order], c, side="right")]
        # sel is sorted by (t, g); group contiguous runs
        st = src[sel]
        dt_ = dst[sel]
        tl = tloc[sel]
        gl = grp[sel]
        key = tl * 2 + gl
        boundaries = np.concatenate([[0], np.where(np.diff(key) != 0)[0] + 1, [len(sel)]])
        for b0, b1 in zip(boundaries[:-1], boundaries[1:]):
            t = int(tl[b0]); g = int(gl[b0])
            n = b1 - b0
            o = int(offs[g][t])
            s_ids = st[b0:b1]
            d_ids = dt_[b0:b1]
            perm = np.argsort(s_ids, kind="stable")  # ascending gather addresses
            s_ids = s_ids[perm]
            d_ids = d_ids[perm]
            idxs[g][o:o + n] = (s_ids - (split if g else 0)).astype(np.int16)
            dsts[g][o:o + n] = (d_ids - c * npc - t * P).astype(np.float32)
        plan.idx_lo.append(_wrap_idx(idxs[0]))
        plan.idx_hi.append(_wrap_idx(idxs[1]))
        plan.dst_lo.append(_wrap_slots(dsts[0]))
        plan.dst_hi.append(_wrap_slots(dsts[1]))
        ic = np.zeros((P, nt), np.float32)
        base = c * npc
        n_own = npc
        icl = invc[base:base + n_own]
        icl = np.concatenate([icl, np.zeros(nt * P - n_own, np.float32)])
        ic[:, :] = icl.reshape(nt, P).T
        plan.invc_tiled.append(np.ascontiguousarray(ic))
    return plan


def stage_inputs(plan: Plan, x, Wl1, Wr1, b1, Wl2, Wr2, b2):
    """Build per-core in_maps (numpy) for the bass program."""
    n, f = x.shape
    hid = plan.n_hid
    ncl = plan.n_class
    zcols = P  # bf16 z row padded to 128 cols = 256B
    x_f32 = np.ascontiguousarray(np.asarray(x, dtype=np.float32))
    x_bf16 = np.ascontiguousarray(x_f32.astype(ml_dtypes.bfloat16))
    wl1 = np.ascontiguousarray(np.asarray(Wl1, np.float32))
    wr1 = np.ascontiguousarray(np.asarray(Wr1, np.float32))
    wl2p = np.zeros((hid, zcols), np.float32)
    wl2p[:, :ncl] = np.asarray(Wl2, np.float32)
    wr2 = np.ascontiguousarray(np.asarray(Wr2, np.float32))
    b1c = np.asarray(b1, np.float32).reshape(hid, 1)
    b2bc = np.broadcast_to(np.asarray(b2, np.float32), (P, ncl)).copy()
    iota = np.broadcast_to(np.arange(P), (P, P)).astype(ml_dtypes.bfloat16)
    ident = np.eye(P, dtype=np.float32)

    in_maps = []
    for c in range(plan.n_cores):
        base = c * plan.npc
        xt = np.zeros((P, plan.nt * P), np.float32)
        xt[:, :plan.npc] = x_f32[base:base + plan.npc].T
        in_maps.append({
            "x_tab": x_bf16,
            "xT_own": xt,
            "idx_lo": plan.idx_lo[c], "idx_hi": plan.idx_hi[c],
            "dst_lo": plan.dst_lo[c].astype(ml_dtypes.bfloat16),
            "dst_hi": plan.dst_hi[c].astype(ml_dtypes.bfloat16),
            "invc": plan.invc_tiled[c],
            "wl1": wl1, "wr1": wr1, "wl2p": wl2p, "wr2": wr2,
            "b1": b1c, "b2": b2bc, "iota": iota, "ident": ident,
        })
    return in_maps


def build_program(plan: Plan, repeats: int = 1, single_core: bool = False,
                  nqueues: int = 1, skip_ag: bool = False):
    n = plan.n_nodes
    f = plan.n_feat
    hid = plan.n_hid
    ncl = plan.n_class
    nt = plan.nt
    npc = plan.npc
    zc = P
    nl, nh = plan.nl, plan.nh
    win = plan.win
    kb = plan.kb
    f32 = mybir.dt.float32
    bf16 = mybir.dt.bfloat16

    nc = bacc.Bacc("TRN2", target_bir_lowering=False, debug=False,
                   enable_asserts=False, num_swdge_queues=nqueues,
                   num_devices=1 if single_core else plan.n_cores)

    x_tab = nc.dram_tensor("x_tab", [n, f], bf16, kind="ExternalInput")
    xT_own = nc.dram_tensor("xT_own", [P, nt * P], f32, kind="ExternalInput")
    idx_lo = nc.dram_tensor("idx_lo", [P, nl // 16], mybir.dt.int16, kind="ExternalInput")
    idx_hi = nc.dram_tensor("idx_hi", [P, nh // 16], mybir.dt.int16, kind="ExternalInput")
    dst_lo = nc.dram_tensor("dst_lo", [P, nl // P], bf16, kind="ExternalInput")
    dst_hi = nc.dram_tensor("dst_hi", [P, nh // P], bf16, kind="ExternalInput")
    invc_d = nc.dram_tensor("invc", [P, nt], f32, kind="ExternalInput")
    wl1_d = nc.dram_tensor("wl1", [f, hid], f32, kind="ExternalInput")
    wr1_d = nc.dram_tensor("wr1", [f, hid], f32, kind="ExternalInput")
    wl2p_d = nc.dram_tensor("wl2p", [hid, zc], f32, kind="ExternalInput")
    wr2_d = nc.dram_tensor("wr2", [hid, ncl], f32, kind="ExternalInput")
    b1_d = nc.dram_tensor("b1", [hid, 1], f32, kind="ExternalInput")
    b2_d = nc.dram_tensor("b2", [P, ncl], f32, kind="ExternalInput")
    iota_d = nc.dram_tensor("iota", [P, P], bf16, kind="ExternalInput")
    ident_d = nc.dram_tensor("ident", [P, P], f32, kind="ExternalInput")
    out_d = nc.dram_tensor("out", [npc, ncl], f32, kind="ExternalOutput")

    with tile.TileContext(nc) as tc:
        nc.gpsimd.load_library(library_config.mlp)
        with tc.tile_pool(name="const", bufs=1) as cp, \
             tc.tile_pool(name="store", bufs=1) as sp, \
             tc.tile_pool(name="msgs", bufs=4) as mp, \
             tc.tile_pool(name="oh", bufs=3) as ohp, \
             tc.tile_pool(name="fin", bufs=2) as fp, \
             tc.tile_pool(name="seg", bufs=2, space="PSUM") as psum_seg, \
             tc.tile_pool(name="paux", bufs=1, space="PSUM") as psum_aux, \
             tc.tile_pool(name="phT", bufs=2, space="PSUM") as psum_h, \
             tc.tile_pool(name="dram", bufs=1, space="DRAM") as dp:

            # ---- constant staging ----
            def load_const(dram, shape, dtype=f32, tag=""):
                t = cp.tile(shape, dtype, tag=tag)
                nc.sync.dma_start(t[:], dram[:])
                return t
            iota_t = load_const(iota_d, [P, P], bf16, tag="iota")
            ident_t = load_const(ident_d, [P, P], tag="ident")
            wl1_t = load_const(wl1_d, [f, hid], tag="wl1")
            wr1_t = load_const(wr1_d, [f, hid], tag="wr1")
            wl2p_t = load_const(wl2p_d, [hid, zc], tag="wl2p")
            wr2_t = load_const(wr2_d, [hid, ncl], tag="wr2")
            b1_t = load_const(b1_d, [hid, 1], tag="b1")
            b2_t = load_const(b2_d, [P, ncl], tag="b2")
            invc_t = load_const(invc_d, [P, nt], tag="invc")
            xT_t = load_const(xT_own, [P, nt * P], tag="xT")
            il_t = load_const(idx_lo, [P, nl // 16], mybir.dt.int16, tag="il")
            ih_t = load_const(idx_hi, [P, nh // 16], mybir.dt.int16, tag="ih")
            dl_t = load_const(dst_lo, [P, nl // P], bf16, tag="dl")
            dh_t = load_const(dst_hi, [P, nh // P], bf16, tag="dh")

            hT_store = sp.tile([P, nt * P], f32, tag="hT_store")  # [hid, node]

            # chunk schedule per stream: list of (tile_idx) per chunk
            budget = plan.budget

            def stream_schedule(g):
                sched = []
                for t in range(nt):
                    sched += [t] * int(budget[t, g])
                return sched

            sched_l = stream_schedule(0)
            sched_h = stream_schedule(1)

            gctr = [0]
            for _rep in range(repeats):
                z_own = dp.tile([npc, zc], bf16, tag=f"z_own{_rep}")
                z_full = dp.tile([n, zc], bf16, addr_space="Shared", tag=f"z_full{_rep}")

                # ================= LAYER 1 =================
                def run_layer(layer):
                    spl = plan.split
                    if layer == 1:
                        tabs = (x_tab[:spl, :], x_tab[spl:, :])
                        mdt, esize = bf16, f
                    else:
                        tabs = (z_full[:spl, :], z_full[spl:, :])
                        mdt, esize = bf16, zc
                    idx_tiles = (il_t, ih_t)
                    dst_tiles = (dl_t, dh_t)
                    totals = (nl, nh)
                    scheds = (sched_l, sched_h)

                    msg_bufs = {}   # (g, w) -> tile
                    oh_bufs = {}    # (g, j) -> tile

                    def ensure_win(g, w):
                        key = (g, w)
                        if key in msg_bufs:
                            return msg_bufs[key]
                        lo = w * win
                        cnt = min(win, totals[g] - lo)
                        mt = mp.tile([P, win // P, esize], mdt, tag="msgs")
                        nc.gpsimd.dma_gather(
                            mt[:, :cnt // P, :], tabs[g], idx_tiles[g][:, lo // 16:(lo + cnt) // 16],
                            cnt, cnt, esize, queue_num=gctr[0] % nqueues)
                        gctr[0] += 1
                        msg_bufs[key] = mt
                        return mt

                    def ensure_oh(g, j):
                        key = (g, j)
                        if key in oh_bufs:
                            return oh_bufs[key]
                        lo = j * kb
                        ncols = min(kb, totals[g] // P - lo)
                        t = ohp.tile([P, kb, P], mdt, tag="oh")
                        dst_sl = dst_tiles[g][:, lo:lo + ncols, None].to_broadcast((P, ncols, P))
                        iota_b = iota_t[:, None, :].to_broadcast((P, ncols, P))
                        nc.vector.tensor_tensor(out=t[:, :ncols, :], in0=dst_sl, in1=iota_b,
                                                op=mybir.AluOpType.is_equal)
                        oh_bufs[key] = t
                        return t

                    chunk_pos = [0, 0]
                    for t in range(nt):
                        pt = psum_seg.tile([P, esize if layer == 2 else f], f32, tag="seg")
                        first = True
                        nchunks = int(budget[t, 0]) + int(budget[t, 1])
                        done = 0
                        for g in (0, 1):
                            for _ in range(int(budget[t, g])):
                                ci = chunk_pos[g]
                                chunk_pos[g] += 1
                                done += 1
                                w, col = divmod(ci * P, win)
                                mt = ensure_win(g, w)
                                oh = ensure_oh(g, ci // kb)
                                if layer == 1:
                                    rhs = mt[:, col // P, :]
                                else:
                                    rhs = mt[:, col // P, 0:64]
                                nc.tensor.matmul(
                                    out=pt[:, 0:64] if layer == 2 else pt[:],
                                    lhsT=oh[:, ci % kb, :], rhs=rhs,
                                    start=first, stop=(done == nchunks))
                                first = False
                        rows = min(P, npc - t * P)
                        if layer == 1:
                            # mean-scale, transpose, dense matmuls, relu
                            aggm = fp.tile([P, f], f32, tag="aggm")
                            if nchunks == 0:
                                nc.vector.memset(aggm[:], 0.0)
                            else:
                                nc.vector.tensor_scalar(
                                    out=aggm[:], in0=pt[:, :f], scalar1=invc_t[:, t:t + 1],
                                    scalar2=None, op0=mybir.AluOpType.mult)
                            paggT = psum_aux.tile([P, P], f32, tag="aggT")
                            nc.tensor.transpose(out=paggT[:], in_=aggm[:], identity=ident_t[:])
                            aggT = fp.tile([P, P], f32, tag="aggT_sb")
                            nc.vector.tensor_copy(out=aggT[:], in_=paggT[:])
                            phT = psum_h.tile([P, P], f32, tag="hT")
                            nc.tensor.matmul(out=phT[:], lhsT=wl1_t[:], rhs=aggT[:],
                                             start=True, stop=False)
                            nc.tensor.matmul(out=phT[:], lhsT=wr1_t[:],
                                             rhs=xT_t[:, t * P:(t + 1) * P],
                                             start=False, stop=True)
                            hT_sl = hT_store[:, t * P:(t + 1) * P]
                            nc.scalar.activation(out=hT_sl, in_=phT[:],
                                                 func=mybir.ActivationFunctionType.Relu,
                                                 bias=b1_t[:], scale=1.0)
                            pz = psum_aux.tile([P, zc], f32, tag="z")
                            nc.tensor.matmul(out=pz[:], lhsT=hT_sl, rhs=wl2p_t[:],
                                             start=True, stop=True)
                            zsb = fp.tile([P, zc], bf16, tag="zsb")
                            nc.vector.tensor_copy(out=zsb[:], in_=pz[:])
                            nc.sync.dma_start(z_own[t * P:t * P + rows, :], zsb[:rows, :])
                        else:
                            s2 = fp.tile([P, ncl], f32, tag="s2")
                            if nchunks == 0:
                                nc.vector.memset(s2[:], 0.0)
                            else:
                                nc.vector.tensor_scalar(
                                    out=s2[:], in0=pt[:, 0:ncl], scalar1=invc_t[:, t:t + 1],
                                    scalar2=None, op0=mybir.AluOpType.mult)
                            po = psum_aux.tile([P, P], f32, tag="aggT")
                            nc.tensor.matmul(out=po[:, 0:ncl], lhsT=hT_store[:, t * P:(t + 1) * P],
                                             rhs=wr2_t[:], start=True, stop=True)
                            ofin = fp.tile([P, ncl], f32, tag="ofin")
                            nc.vector.tensor_add(out=ofin[:], in0=po[:, 0:ncl], in1=s2[:])
                            nc.vector.tensor_add(out=ofin[:], in0=ofin[:], in1=b2_t[:, :ncl])
                            nc.sync.dma_start(out_d[t * P:t * P + rows, :], ofin[:rows, :])

                run_layer(1)
                if not skip_ag:
                    nc.gpsimd.collective_compute(
                        "AllGather", mybir.AluOpType.bypass,
                        replica_groups=[list(range(plan.n_cores))],
                        ins=[z_own[:]], outs=[z_full[:]])
                run_layer(2)

    nc.compile()
    return nc


import numpy as np
import jax
from jax.sharding import Mesh, PartitionSpec
from jax.experimental.shard_map import shard_map
import concourse.mybir as mybir
import concourse.bass2jax as bass2jax
from concourse.bass2jax import _bass_exec_p, partition_id_tensor, install_neuronx_cc_hook


class SpmdRunner:
    def __init__(self, nc, n_cores: int):
        install_neuronx_cc_hook()
        self.nc = nc
        self.n_cores = n_cores
        partition_name = nc.partition_id_tensor.name if nc.partition_id_tensor else None
        in_names, out_names, out_avals = [], [], []
        zero_outs = []
        for alloc in nc.m.functions[0].allocations:
            if not isinstance(alloc, mybir.MemoryLocationSet):
                continue
            name = alloc.memorylocations[0].name
            if alloc.kind == "ExternalInput":
                if name != partition_name:
                    in_names.append(name)
            elif alloc.kind == "ExternalOutput":
                shape = tuple(alloc.tensor_shape)
                dtype = mybir.dt.np(alloc.dtype)
                out_names.append(name)
                out_avals.append(jax.core.ShapedArray(shape, dtype))
                zero_outs.append(np.zeros(shape, dtype))
        self.in_names = list(in_names)
        self.out_names = out_names
        self.out_avals = out_avals
        self.zero_outs = zero_outs
        n_params = len(in_names)
        all_in_names = list(in_names) + list(out_names)
        if partition_name is not None:
            all_in_names.append(partition_name)

        def _body(*args):
            operands = list(args)
            if partition_name is not None:
                operands.append(partition_id_tensor())
            outs = _bass_exec_p.bind(
                *operands,
                out_avals=tuple(out_avals),
                in_names=tuple(all_in_names),
                out_names=tuple(out_names),
                lowering_input_output_aliases=(),
                sim_require_finite=False,
                sim_require_nnan=False,
                nc=nc,
            )
            return tuple(outs)

        devices = jax.devices()[:n_cores]
        assert len(devices) == n_cores
        self.mesh = Mesh(np.asarray(devices), ("core",))
        in_specs = (PartitionSpec("core"),) * (n_params + len(out_names))
        out_specs = (PartitionSpec("core"),) * len(out_names)
        self.fn = jax.jit(
            shard_map(_body, mesh=self.mesh, in_specs=in_specs,
                      out_specs=out_specs, check_rep=False),
            keep_unused=True,
        )
        self._dev_args = None

    def stage(self, in_maps):
        """Concatenate per-core inputs and device_put once."""
        n = self.n_cores
        concat_in = [
            np.concatenate([np.asarray(in_maps[c][name]) for c in range(n)], axis=0)
            for name in self.in_names
        ]
        concat_zeros = [
            np.zeros((n * z.shape[0], *z.shape[1:]), z.dtype) for z in self.zero_outs
        ]
        from jax.sharding import NamedSharding
        sh = NamedSharding(self.mesh, PartitionSpec("core"))
        self._dev_args = [jax.device_put(a, sh) for a in concat_in + concat_zeros]
        return self

    def run(self):
        return self.fn(*self._dev_args)

    def run_blocking(self):
        out = self.fn(*self._dev_args)
        jax.block_until_ready(out)
        return out

    def results(self, out_arrs):
        n = self.n_cores
        return [
            {name: np.asarray(out_arrs[i]).reshape(n, *self.out_avals[i].shape)[c]
             for i, name in enumerate(self.out_names)}
            for c in range(n)
        ]


# ---------------- self-contained entry point ----------------
_CACHE = {}

def kernel(**inputs):
    import numpy as _np
    x = _np.asarray(inputs["x"], dtype=_np.float32)
    edge_index = _np.asarray(inputs["edge_index"])
    Wl1 = _np.asarray(inputs["Wl1"], dtype=_np.float32)
    Wr1 = _np.asarray(inputs["Wr1"], dtype=_np.float32)
    b1 = _np.asarray(inputs["b1"], dtype=_np.float32)
    Wl2 = _np.asarray(inputs["Wl2"], dtype=_np.float32)
    Wr2 = _np.asarray(inputs["Wr2"], dtype=_np.float32)
    b2 = _np.asarray(inputs["b2"], dtype=_np.float32)
    N, F = x.shape
    H = Wl1.shape[1]
    C = Wl2.shape[1]
    import hashlib
    eh = hashlib.md5(edge_index.tobytes()).hexdigest()
    key = ("plan", N, F, H, C, edge_index.shape[1], eh)
    if key not in _CACHE:
        plan = make_plan(edge_index, N, F, H, C, 8, win=1024, kb=16)
        nc = build_program(plan)
        runner = SpmdRunner(nc, 8)
        _CACHE[key] = (plan, runner)
    plan, runner = _CACHE[key]
    in_maps = stage_inputs(plan, x, Wl1, Wr1, b1, Wl2, Wr2, b2)
    runner.stage(in_maps)
    out_arrs = runner.run_blocking()
    results = runner.results(out_arrs)
    out = _np.concatenate([results[c]["out"] for c in range(8)], axis=0)
    return out[:N].astype(_np.float32)

